# revision 40
# baseline (speedup 1.0000x reference)
"""Trainium2 Bass kernel for nn_ActorCriticReadOut.

Problem (hardcoded shapes): B=64 graphs x 512 nodes, D=512, H=2048.
  z[g, n]  = where(mask, MLP_3(x)[g*512+n], -inf)        -> [64, 512]
  v[g]     = MLP_vf(mean_n x[g*512+n])                   -> [64, 1]

Sharding: data-parallel over graphs. 8 cores x 8 graphs (4096 nodes) each;
MLP weights replicated. Everything is local per core; host concatenates.

Per-core device algorithm (transposed-activation layout, bf16 compute
with f32 PSUM accumulation):
  phase A (traced first): stream x once (8MB), PE-transpose every
      128x128 tile, DVE-copy psum -> resident xT (bf16, 32KB/part for
      all 8 graphs), and DVE-reduce each xT chunk over its free (node)
      axis to build the segment means directly in transposed layout.
      This unblocks the value MLP ~35us in, so its 20MB of weights
      stream overlapped with the main phase.
  value MLP: all weights streamed (f32 stage -> bf16 cast), no
      residency. L1: k-outer, one-shot matmuls into a packed psum bank
      + DVE accumulate (matmul start=True clears has_written for the
      WHOLE bank, so interleaved accumulation regions are illegal).
      L2: row-form (stationary = h1v column chunk [128, 8], moving =
      weight rows, N=512) -> h2 rows [8, 2048] accumulated in SBUF.
      L3 on DVE: in-place row * w3-row multiply, reduce over free, +vb3.
  main MLP (per 512-node block == one graph, no DMA in the loop):
      L1/L2 bf16 (resident bf16 W1+W2), relu+bias fused into the
      PSUM->SBUF copy on ScalarE; L3 (W3 column stationary, M=1)
      interleaved into L2's m-loop; epilogue adds b3 + additive mask
      row (0/-inf) and DMAs the z row out.
"""

import threading
from contextlib import ExitStack

import numpy as np

import concourse.tile as tile
from concourse import bacc, mybir
from concourse.bass_utils import run_bass_kernel_spmd
from concourse.masks import make_identity

F32 = mybir.dt.float32
BF16 = mybir.dt.bfloat16
AF = mybir.ActivationFunctionType

N_CORES = 8
B_LOC = 8            # graphs per core
NBLK = 512           # nodes per graph (= node block)
D = 512
H = 2048
NODES = B_LOC * NBLK  # 4096
KD = D // 128         # 4 contraction chunks for D
KH = H // 128         # 16 contraction chunks for H
NSUB = NBLK // 128    # 4 node sub-chunks per block
NH = H // 512         # 4 n-slices of H for row-form matmuls


def _build():
    nc = bacc.Bacc(name="actor_critic_readout")

    x_ext = nc.declare_dram_parameter("x", [NODES, D], F32, isOutput=False)
    mb_ext = nc.declare_dram_parameter("mb", [B_LOC, NBLK], F32, isOutput=False)
    w1_ext = nc.declare_dram_parameter("w1", [D, H], F32, isOutput=False)
    b1_ext = nc.declare_dram_parameter("b1", [H], F32, isOutput=False)
    w2_ext = nc.declare_dram_parameter("w2", [H, H], F32, isOutput=False)
    b2_ext = nc.declare_dram_parameter("b2", [H], F32, isOutput=False)
    w3_ext = nc.declare_dram_parameter("w3", [H, 1], F32, isOutput=False)
    b3_ext = nc.declare_dram_parameter("b3", [1], F32, isOutput=False)
    vw1_ext = nc.declare_dram_parameter("vw1", [D, H], F32, isOutput=False)
    vb1_ext = nc.declare_dram_parameter("vb1", [H], F32, isOutput=False)
    vw2_ext = nc.declare_dram_parameter("vw2", [H, H], F32, isOutput=False)
    vb2_ext = nc.declare_dram_parameter("vb2", [H], F32, isOutput=False)
    vw3_ext = nc.declare_dram_parameter("vw3", [H, 1], F32, isOutput=False)
    vb3_ext = nc.declare_dram_parameter("vb3", [1], F32, isOutput=False)
    z_ext = nc.declare_dram_parameter("z", [B_LOC, NBLK], F32, isOutput=True)
    v_ext = nc.declare_dram_parameter("v", [B_LOC, 1], F32, isOutput=True)

    with ExitStack() as ctx:
        tc = ctx.enter_context(tile.TileContext(nc))
        const = ctx.enter_context(tc.tile_pool(name="const", bufs=1))
        wres = ctx.enter_context(tc.tile_pool(name="wres", bufs=1))
        wstage = ctx.enter_context(tc.tile_pool(name="wstage", bufs=3))
        xs_pool = ctx.enter_context(tc.tile_pool(name="xs", bufs=6))
        xta_pool = ctx.enter_context(tc.tile_pool(name="xta", bufs=1))
        h1_pool = ctx.enter_context(tc.tile_pool(name="h1", bufs=17))
        h2_pool = ctx.enter_context(tc.tile_pool(name="h2", bufs=2))
        sm_pool = ctx.enter_context(tc.tile_pool(name="sm", bufs=2))
        vf_pool = ctx.enter_context(tc.tile_pool(name="vf", bufs=1))
        p_l1 = ctx.enter_context(tc.tile_pool(name="p_l1", bufs=3, space="PSUM"))
        # "aux" (1 bank): per-block z rows. "pv" (1 bank): one-shot vf banks.
        p_aux = ctx.enter_context(tc.tile_pool(name="p_aux", bufs=1, space="PSUM"))
        p_vf = ctx.enter_context(tc.tile_pool(name="p_vf", bufs=1, space="PSUM"))

        ident = const.tile([128, 128], BF16, name="ident")
        make_identity(nc, ident)

        # ---- phase A: x -> xT (bf16 resident) + segment sums -----------
        hmsum = [
            const.tile([128, B_LOC], F32, name=f"hmsum_{k}", tag=f"hmsum_{k}")
            for k in range(KD)
        ]
        xta = [[None] * KD for _ in range(B_LOC)]
        with tc.tile_pool(name="p_tp", bufs=2, space="PSUM") as p_tp:
            for b in range(B_LOC):
                xs_tiles = []
                for j in range(NSUB):
                    xs = xs_pool.tile([128, D], F32, name=f"x_{b}_{j}", tag="xs")
                    row = b * NBLK + j * 128
                    nc.sync.dma_start(xs[:], x_ext[row : row + 128, :])
                    xb = xs_pool.tile([128, D], BF16, name=f"xb_{b}_{j}", tag="xb")
                    nc.scalar.copy(xb[:], xs[:])
                    xs_tiles.append(xb)
                for k in range(KD):
                    tp = p_tp.tile([128, NBLK], BF16, name=f"tp_{b}_{k}", tag="tp")
                    for j in range(NSUB):
                        nc.tensor.transpose(
                            tp[:, j * 128 : (j + 1) * 128],
                            xs_tiles[j][:, k * 128 : (k + 1) * 128],
                            ident[:],
                        )
                    t = xta_pool.tile(
                        [128, NBLK], BF16, name=f"xt_{b}_{k}", tag=f"xt_{b}_{k}"
                    )
                    nc.vector.tensor_copy(t[:], tp[:])
                    xta[b][k] = t
                    nc.vector.reduce_sum(
                        hmsum[k][:, b : b + 1], t[:], axis=mybir.AxisListType.X
                    )
        p_l2 = ctx.enter_context(tc.tile_pool(name="p_l2", bufs=3, space="PSUM"))
        # h_meanT chunks, bf16, scale 1/NBLK
        hmb = []
        for k in range(KD):
            t = const.tile([128, B_LOC], BF16, name=f"hmb_{k}", tag=f"hmb_{k}")
            nc.scalar.activation(t[:], hmsum[k][:], AF.Copy, scale=1.0 / NBLK)
            hmb.append(t)

        # ---- small constants -------------------------------------------
        b1t = const.tile([128, KH], F32, name="b1t")
        nc.sync.dma_start(b1t[:], b1_ext.rearrange("(m p) -> p m", p=128))
        b2t = const.tile([128, KH], F32, name="b2t")
        nc.sync.dma_start(b2t[:], b2_ext.rearrange("(m p) -> p m", p=128))
        vb1t = const.tile([128, KH], F32, name="vb1t")
        nc.sync.dma_start(vb1t[:], vb1_ext.rearrange("(m p) -> p m", p=128))
        b3t = const.tile([1, 1], F32, name="b3t")
        nc.sync.dma_start(b3t[:], b3_ext[None, :])
        vb3col = const.tile([B_LOC, 1], F32, name="vb3col")
        for g in range(B_LOC):
            nc.sync.dma_start(vb3col[g : g + 1, :], vb3_ext[None, :])
        w3f = const.tile([128, KH], F32, name="w3f")
        nc.sync.dma_start(w3f[:], w3_ext.rearrange("(k p) one -> p (k one)", p=128))
        w3b = const.tile([128, KH], BF16, name="w3b")
        nc.vector.tensor_copy(w3b[:], w3f[:])

        # ---- resident weights (casts on ScalarE to keep DVE clear) -----
        w1 = []
        for k in range(KD):
            stage = wstage.tile([128, H], F32, name=f"w1s_{k}", tag="wstage")
            nc.sync.dma_start(stage[:], w1_ext[k * 128 : (k + 1) * 128, :])
            t = wres.tile([128, H], BF16, name=f"w1_{k}", tag=f"w1_{k}")
            nc.scalar.copy(t[:], stage[:])
            w1.append(t)
        w2 = []
        for k in range(KH):
            stage = wstage.tile([128, H], F32, name=f"w2s_{k}", tag="wstage")
            nc.sync.dma_start(stage[:], w2_ext[k * 128 : (k + 1) * 128, :])
            t = wres.tile([128, H], BF16, name=f"w2_{k}", tag=f"w2_{k}")
            nc.scalar.copy(t[:], stage[:])
            w2.append(t)

        # ---- vf L1: streamed vw1, k-outer, SBUF accumulate -------------
        h1acc = const.tile([128, KH, B_LOC], F32, name="h1acc")
        for k in range(KD):
            stage = wstage.tile([128, H], F32, name=f"vw1s_{k}", tag="wstage")
            nc.sync.dma_start(stage[:], vw1_ext[k * 128 : (k + 1) * 128, :])
            sbf = wstage.tile([128, H], BF16, name=f"vw1b_{k}", tag="wstageb")
            nc.vector.tensor_copy(sbf[:], stage[:])
            pk = p_vf.tile([128, KH, B_LOC], F32, name=f"pv1_{k}", tag="pv")
            for m in range(KH):
                nc.tensor.matmul(
                    pk[:, m, :],
                    sbf[:, m * 128 : (m + 1) * 128],
                    hmb[k][:],
                    start=True,
                    stop=True,
                )
            if k == 0:
                nc.vector.tensor_copy(h1acc[:], pk[:])
            else:
                nc.vector.tensor_add(h1acc[:], h1acc[:], pk[:])
        h1v = []
        for m in range(KH):
            t = vf_pool.tile([128, B_LOC], BF16, name=f"h1v_{m}", tag=f"h1v_{m}")
            nc.scalar.activation(t[:], h1acc[:, m, :], AF.Relu, bias=vb1t[:, m : m + 1])
            h1v.append(t)

        # ---- vf L2: row-form, streamed vw2, SBUF accumulate ------------
        vacc2 = const.tile([B_LOC, H], F32, name="vacc2")

        def vf_l2_chunk(k):
            stage = wstage.tile([128, H], F32, name=f"vw2s_{k}", tag="wstage")
            nc.sync.dma_start(stage[:], vw2_ext[k * 128 : (k + 1) * 128, :])
            sbf = wstage.tile([128, H], BF16, name=f"vw2b_{k}", tag="wstageb")
            nc.vector.tensor_copy(sbf[:], stage[:])
            for n in range(NH):
                pb = p_vf.tile([B_LOC, 512], F32, name=f"pv2_{k}_{n}", tag="pv")
                nc.tensor.matmul(
                    pb[:],
                    h1v[k][:],
                    sbf[:, n * 512 : (n + 1) * 512],
                    start=True,
                    stop=True,
                )
                sl = vacc2[:, n * 512 : (n + 1) * 512]
                if k == 0:
                    nc.vector.tensor_copy(sl, pb[:])
                else:
                    nc.vector.tensor_add(sl, sl, pb[:])

        # ---- main MLP: one graph (=512-node block) at a time -----------
        for b in range(B_LOC):
            h1 = []
            for m in range(KH):
                ps = p_l1.tile([128, NBLK], F32, name=f"pl1_{b}_{m}", tag="l1")
                for k in range(KD):
                    nc.tensor.matmul(
                        ps[:],
                        w1[k][:, m * 128 : (m + 1) * 128],
                        xta[b][k][:],
                        start=(k == 0),
                        stop=(k == KD - 1),
                    )
                t = h1_pool.tile([128, NBLK], BF16, name=f"h1_{b}_{m}", tag="h1")
                nc.scalar.activation(t[:], ps[:], AF.Relu, bias=b1t[:, m : m + 1])
                h1.append(t)

            zp = p_aux.tile([1, NBLK], F32, name=f"zp_{b}", tag="aux")
            h2_prev = None
            for m in range(KH):
                ps = p_l2.tile([128, NBLK], F32, name=f"pl2_{b}_{m}", tag="l2")
                for k in range(KH):
                    nc.tensor.matmul(
                        ps[:],
                        w2[k][:, m * 128 : (m + 1) * 128],
                        h1[k][:],
                        start=(k == 0),
                        stop=(k == KH - 1),
                    )
                h2 = h2_pool.tile([128, NBLK], BF16, name=f"h2_{b}_{m}", tag="h2")
                nc.scalar.activation(h2[:], ps[:], AF.Relu, bias=b2t[:, m : m + 1])
                # z matmul for the PREVIOUS chunk: its h2 is long since
                # written, so the PE never waits on the ScalarE relu here.
                if h2_prev is not None:
                    nc.tensor.matmul(
                        zp[:], w3b[:, m - 1 : m], h2_prev[:],
                        start=(m == 1), stop=False,
                    )
                h2_prev = h2
            nc.tensor.matmul(
                zp[:], w3b[:, KH - 1 : KH], h2_prev[:],
                start=False, stop=True,
            )

            mb = sm_pool.tile([1, NBLK], F32, name=f"mb_{b}", tag="mb", bufs=1)
            nc.sync.dma_start(mb[:], mb_ext[b : b + 1, :])
            zr = sm_pool.tile([1, NBLK], F32, name=f"zr_{b}", tag="zr", bufs=1)
            nc.scalar.activation(zr[:], zp[:], AF.Identity, bias=b3t[:])
            nc.vector.tensor_add(zr[:], zr[:], mb[:])
            nc.sync.dma_start(z_ext[b : b + 1, :], zr[:])

            vf_l2_chunk(2 * b)
            vf_l2_chunk(2 * b + 1)

        # ---- vf tail: bias+relu rows, L3 dot on DVE --------------------
        # row constants borrow wstage slots (free once the vw2 stream ends)
        vb2row = wstage.tile([B_LOC, H], F32, name="vb2row", tag="wstage")
        vw3row = wstage.tile([B_LOC, H], F32, name="vw3row", tag="wstage")
        for g in range(B_LOC):
            nc.sync.dma_start(vb2row[g : g + 1, :], vb2_ext[None, :])
            nc.sync.dma_start(vw3row[g : g + 1, :], vw3_ext.rearrange("h one -> one h"))
        nc.vector.tensor_add(vacc2[:], vacc2[:], vb2row[:])
        nc.scalar.activation(vacc2[:], vacc2[:], AF.Relu)
        nc.vector.tensor_mul(vacc2[:], vacc2[:], vw3row[:])
        v0 = sm_pool.tile([B_LOC, 1], F32, name="v0", tag="v_sb", bufs=1)
        nc.vector.reduce_sum(v0[:], vacc2[:], axis=mybir.AxisListType.X)
        v_sb = sm_pool.tile([B_LOC, 1], F32, name="v_sb", tag="v_sb2", bufs=1)
        nc.scalar.activation(v_sb[:], v0[:], AF.Identity, bias=vb3col[:])
        nc.sync.dma_start(v_ext[:], v_sb[:])

    nc.compile()
    return nc


_CACHE = threading.Lock()
_NC = None


def _get_nc():
    global _NC
    with _CACHE:
        if _NC is None:
            _NC = _build()
    return _NC


def kernel(**inputs):
    x = np.ascontiguousarray(np.asarray(inputs["x"], dtype=np.float32))
    mask = np.asarray(inputs["action_mask"])
    mb = np.where(mask, np.float32(0.0), np.float32(-np.inf)).astype(np.float32)
    mb = np.ascontiguousarray(mb.reshape(N_CORES * B_LOC, NBLK))

    def f32(name):
        return np.ascontiguousarray(np.asarray(inputs[name], dtype=np.float32))

    shared = {
        "w1": f32("mlp_W1"), "b1": f32("mlp_b1"),
        "w2": f32("mlp_W2"), "b2": f32("mlp_b2"),
        "w3": f32("mlp_W3"), "b3": f32("mlp_b3"),
        "vw1": f32("vf_W1"), "vb1": f32("vf_b1"),
        "vw2": f32("vf_W2"), "vb2": f32("vf_b2"),
        "vw3": f32("vf_W3"), "vb3": f32("vf_b3"),
    }
    in_maps = []
    for c in range(N_CORES):
        m = dict(shared)
        m["x"] = x[c * NODES : (c + 1) * NODES]
        m["mb"] = mb[c * B_LOC : (c + 1) * B_LOC]
        in_maps.append(m)

    nc = _get_nc()
    res = run_bass_kernel_spmd(nc, in_maps, core_ids=list(range(N_CORES)))
    z = np.concatenate([res.results[c]["z"] for c in range(N_CORES)], axis=0)
    v = np.concatenate([res.results[c]["v"] for c in range(N_CORES)], axis=0)
    return z, v


# revision 41
# speedup vs baseline: 1.1688x; 1.1688x over previous
"""Trainium2 Bass kernel for nn_ActorCriticReadOut.

Problem (hardcoded shapes): B=64 graphs x 512 nodes, D=512, H=2048.
  z[g, n]  = where(mask, MLP_3(x)[g*512+n], -inf)        -> [64, 512]
  v[g]     = MLP_vf(mean_n x[g*512+n])                   -> [64, 1]

Sharding: data-parallel over graphs. 8 cores x 8 graphs (4096 nodes) each;
MLP weights replicated. Everything is local per core; host concatenates.

Per-core device algorithm (transposed-activation layout, bf16 compute
with f32 PSUM accumulation):
  phase A (traced first): stream x once (8MB), PE-transpose every
      128x128 tile, DVE-copy psum -> resident xT (bf16, 32KB/part for
      all 8 graphs), and DVE-reduce each xT chunk over its free (node)
      axis to build the segment means directly in transposed layout.
      This unblocks the value MLP ~35us in, so its 20MB of weights
      stream overlapped with the main phase.
  value MLP: all weights streamed (f32 stage -> bf16 cast), no
      residency. L1: k-outer, one-shot matmuls into a packed psum bank
      + DVE accumulate (matmul start=True clears has_written for the
      WHOLE bank, so interleaved accumulation regions are illegal).
      L2: row-form (stationary = h1v column chunk [128, 8], moving =
      weight rows, N=512) -> h2 rows [8, 2048] accumulated in SBUF.
      L3 on DVE: in-place row * w3-row multiply, reduce over free, +vb3.
  main MLP (per 512-node block == one graph, no DMA in the loop):
      L1/L2 bf16 (resident bf16 W1+W2), relu+bias fused into the
      PSUM->SBUF copy on ScalarE; L3 (W3 column stationary, M=1)
      interleaved into L2's m-loop; epilogue adds b3 + additive mask
      row (0/-inf) and DMAs the z row out.
"""

import threading
from contextlib import ExitStack

import numpy as np

import concourse.tile as tile
from concourse import bacc, mybir
from concourse.bass_utils import run_bass_kernel_spmd
from concourse.masks import make_identity

F32 = mybir.dt.float32
BF16 = mybir.dt.bfloat16
AF = mybir.ActivationFunctionType

N_CORES = 8
B_LOC = 8            # graphs per core
NBLK = 512           # nodes per graph (= node block)
D = 512
H = 2048
NODES = B_LOC * NBLK  # 4096
KD = D // 128         # 4 contraction chunks for D
KH = H // 128         # 16 contraction chunks for H
NSUB = NBLK // 128    # 4 node sub-chunks per block
NH = H // 512         # 4 n-slices of H for row-form matmuls


def _build():
    nc = bacc.Bacc(name="actor_critic_readout")

    x_ext = nc.declare_dram_parameter("x", [NODES, D], F32, isOutput=False)
    mb_ext = nc.declare_dram_parameter("mb", [B_LOC, NBLK], F32, isOutput=False)
    w1_ext = nc.declare_dram_parameter("w1", [D, H], F32, isOutput=False)
    b1_ext = nc.declare_dram_parameter("b1", [H], F32, isOutput=False)
    w2_ext = nc.declare_dram_parameter("w2", [H, H], F32, isOutput=False)
    b2_ext = nc.declare_dram_parameter("b2", [H], F32, isOutput=False)
    w3_ext = nc.declare_dram_parameter("w3", [H, 1], F32, isOutput=False)
    b3_ext = nc.declare_dram_parameter("b3", [1], F32, isOutput=False)
    vw1_ext = nc.declare_dram_parameter("vw1", [D, H], F32, isOutput=False)
    vb1_ext = nc.declare_dram_parameter("vb1", [H], F32, isOutput=False)
    vw2_ext = nc.declare_dram_parameter("vw2", [H, H], F32, isOutput=False)
    vb2_ext = nc.declare_dram_parameter("vb2", [H], F32, isOutput=False)
    vw3_ext = nc.declare_dram_parameter("vw3", [H, 1], F32, isOutput=False)
    vb3_ext = nc.declare_dram_parameter("vb3", [1], F32, isOutput=False)
    z_ext = nc.declare_dram_parameter("z", [B_LOC, NBLK], F32, isOutput=True)
    v_ext = nc.declare_dram_parameter("v", [B_LOC, 1], F32, isOutput=True)

    with ExitStack() as ctx:
        tc = ctx.enter_context(tile.TileContext(nc))
        const = ctx.enter_context(tc.tile_pool(name="const", bufs=1))
        wres = ctx.enter_context(tc.tile_pool(name="wres", bufs=1))
        wstage = ctx.enter_context(tc.tile_pool(name="wstage", bufs=3))
        xs_pool = ctx.enter_context(tc.tile_pool(name="xs", bufs=6))
        xta_pool = ctx.enter_context(tc.tile_pool(name="xta", bufs=1))
        h1_pool = ctx.enter_context(tc.tile_pool(name="h1", bufs=17))
        h2_pool = ctx.enter_context(tc.tile_pool(name="h2", bufs=2))
        sm_pool = ctx.enter_context(tc.tile_pool(name="sm", bufs=2))
        vf_pool = ctx.enter_context(tc.tile_pool(name="vf", bufs=1))
        p_l1 = ctx.enter_context(tc.tile_pool(name="p_l1", bufs=3, space="PSUM"))
        # "aux" (1 bank): per-block z rows. "pv" (1 bank): one-shot vf banks.
        p_aux = ctx.enter_context(tc.tile_pool(name="p_aux", bufs=1, space="PSUM"))
        p_vf = ctx.enter_context(tc.tile_pool(name="p_vf", bufs=2, space="PSUM"))

        ident = const.tile([128, 128], BF16, name="ident")
        make_identity(nc, ident)

        # ---- phase A: x -> xT (bf16 resident) + segment sums -----------
        hmsum = [
            const.tile([128, B_LOC], F32, name=f"hmsum_{k}", tag=f"hmsum_{k}")
            for k in range(KD)
        ]
        xta = [[None] * KD for _ in range(B_LOC)]
        with tc.tile_pool(name="p_tp", bufs=2, space="PSUM") as p_tp:
            for b in range(B_LOC):
                xs_tiles = []
                for j in range(NSUB):
                    xs = xs_pool.tile([128, D], F32, name=f"x_{b}_{j}", tag="xs")
                    row = b * NBLK + j * 128
                    nc.sync.dma_start(xs[:], x_ext[row : row + 128, :])
                    xb = xs_pool.tile([128, D], BF16, name=f"xb_{b}_{j}", tag="xb")
                    nc.scalar.copy(xb[:], xs[:])
                    xs_tiles.append(xb)
                for k in range(KD):
                    tp = p_tp.tile([128, NBLK], BF16, name=f"tp_{b}_{k}", tag="tp")
                    for j in range(NSUB):
                        nc.tensor.transpose(
                            tp[:, j * 128 : (j + 1) * 128],
                            xs_tiles[j][:, k * 128 : (k + 1) * 128],
                            ident[:],
                        )
                    t = xta_pool.tile(
                        [128, NBLK], BF16, name=f"xt_{b}_{k}", tag=f"xt_{b}_{k}"
                    )
                    nc.vector.tensor_copy(t[:], tp[:])
                    xta[b][k] = t
                    nc.vector.reduce_sum(
                        hmsum[k][:, b : b + 1], t[:], axis=mybir.AxisListType.X
                    )
        p_l2 = ctx.enter_context(tc.tile_pool(name="p_l2", bufs=2, space="PSUM"))
        # h_meanT chunks, bf16, scale 1/NBLK
        hmb = []
        for k in range(KD):
            t = const.tile([128, B_LOC], BF16, name=f"hmb_{k}", tag=f"hmb_{k}")
            nc.scalar.activation(t[:], hmsum[k][:], AF.Copy, scale=1.0 / NBLK)
            hmb.append(t)

        # ---- small constants -------------------------------------------
        b1t = const.tile([128, KH], F32, name="b1t")
        nc.sync.dma_start(b1t[:], b1_ext.rearrange("(m p) -> p m", p=128))
        b2t = const.tile([128, KH], F32, name="b2t")
        nc.sync.dma_start(b2t[:], b2_ext.rearrange("(m p) -> p m", p=128))
        vb1t = const.tile([128, KH], F32, name="vb1t")
        nc.sync.dma_start(vb1t[:], vb1_ext.rearrange("(m p) -> p m", p=128))
        b3t = const.tile([1, 1], F32, name="b3t")
        nc.sync.dma_start(b3t[:], b3_ext[None, :])
        vb3col = const.tile([B_LOC, 1], F32, name="vb3col")
        for g in range(B_LOC):
            nc.sync.dma_start(vb3col[g : g + 1, :], vb3_ext[None, :])
        w3f = const.tile([128, KH], F32, name="w3f")
        nc.sync.dma_start(w3f[:], w3_ext.rearrange("(k p) one -> p (k one)", p=128))
        w3b = const.tile([128, KH], BF16, name="w3b")
        nc.vector.tensor_copy(w3b[:], w3f[:])

        # ---- resident weights (casts on ScalarE to keep DVE clear) -----
        w1 = []
        for k in range(KD):
            stage = wstage.tile([128, H], F32, name=f"w1s_{k}", tag="wstage")
            nc.sync.dma_start(stage[:], w1_ext[k * 128 : (k + 1) * 128, :])
            t = wres.tile([128, H], BF16, name=f"w1_{k}", tag=f"w1_{k}")
            nc.scalar.copy(t[:], stage[:])
            w1.append(t)
        w2 = []
        for k in range(KH):
            stage = wstage.tile([128, H], F32, name=f"w2s_{k}", tag="wstage")
            nc.sync.dma_start(stage[:], w2_ext[k * 128 : (k + 1) * 128, :])
            t = wres.tile([128, H], BF16, name=f"w2_{k}", tag=f"w2_{k}")
            nc.scalar.copy(t[:], stage[:])
            w2.append(t)

        # ---- vf L1: streamed vw1, k-outer, SBUF accumulate -------------
        h1acc = const.tile([128, KH, B_LOC], F32, name="h1acc")
        for k in range(KD):
            stage = wstage.tile([128, H], F32, name=f"vw1s_{k}", tag="wstage")
            nc.sync.dma_start(stage[:], vw1_ext[k * 128 : (k + 1) * 128, :])
            sbf = wstage.tile([128, H], BF16, name=f"vw1b_{k}", tag="wstageb")
            nc.vector.tensor_copy(sbf[:], stage[:])
            pk = p_vf.tile([128, KH, B_LOC], F32, name=f"pv1_{k}", tag="pv")
            for m in range(KH):
                nc.tensor.matmul(
                    pk[:, m, :],
                    sbf[:, m * 128 : (m + 1) * 128],
                    hmb[k][:],
                    start=True,
                    stop=True,
                )
            if k == 0:
                nc.vector.tensor_copy(h1acc[:], pk[:])
            else:
                nc.vector.tensor_add(h1acc[:], h1acc[:], pk[:])
        h1v = []
        for m in range(KH):
            t = vf_pool.tile([128, B_LOC], BF16, name=f"h1v_{m}", tag=f"h1v_{m}")
            nc.scalar.activation(t[:], h1acc[:, m, :], AF.Relu, bias=vb1t[:, m : m + 1])
            h1v.append(t)

        # ---- vf L2: row-form, streamed vw2, SBUF accumulate ------------
        vacc2 = const.tile([B_LOC, H], F32, name="vacc2")

        def vf_l2_chunk(k):
            stage = wstage.tile([128, H], F32, name=f"vw2s_{k}", tag="wstage")
            nc.sync.dma_start(stage[:], vw2_ext[k * 128 : (k + 1) * 128, :])
            sbf = wstage.tile([128, H], BF16, name=f"vw2b_{k}", tag="wstageb")
            nc.vector.tensor_copy(sbf[:], stage[:])
            for n in range(NH):
                pb = p_vf.tile([B_LOC, 512], F32, name=f"pv2_{k}_{n}", tag="pv")
                nc.tensor.matmul(
                    pb[:],
                    h1v[k][:],
                    sbf[:, n * 512 : (n + 1) * 512],
                    start=True,
                    stop=True,
                )
                sl = vacc2[:, n * 512 : (n + 1) * 512]
                if k == 0:
                    nc.vector.tensor_copy(sl, pb[:])
                else:
                    nc.vector.tensor_add(sl, sl, pb[:])

        # ---- main MLP: one graph (=512-node block) at a time -----------
        for b in range(B_LOC):
            h1 = []
            for m in range(KH):
                ps = p_l1.tile([128, NBLK], F32, name=f"pl1_{b}_{m}", tag="l1")
                for k in range(KD):
                    nc.tensor.matmul(
                        ps[:],
                        w1[k][:, m * 128 : (m + 1) * 128],
                        xta[b][k][:],
                        start=(k == 0),
                        stop=(k == KD - 1),
                    )
                t = h1_pool.tile([128, NBLK], BF16, name=f"h1_{b}_{m}", tag="h1")
                nc.scalar.activation(t[:], ps[:], AF.Relu, bias=b1t[:, m : m + 1])
                h1.append(t)

            zp = p_aux.tile([1, NBLK], F32, name=f"zp_{b}", tag="aux")
            h2_prev = None
            for m in range(KH):
                ps = p_l2.tile([128, NBLK], F32, name=f"pl2_{b}_{m}", tag="l2")
                for k in range(KH):
                    nc.tensor.matmul(
                        ps[:],
                        w2[k][:, m * 128 : (m + 1) * 128],
                        h1[k][:],
                        start=(k == 0),
                        stop=(k == KH - 1),
                    )
                h2 = h2_pool.tile([128, NBLK], BF16, name=f"h2_{b}_{m}", tag="h2")
                nc.scalar.activation(h2[:], ps[:], AF.Relu, bias=b2t[:, m : m + 1])
                # z matmul for the PREVIOUS chunk: its h2 is long since
                # written, so the PE never waits on the ScalarE relu here.
                if h2_prev is not None:
                    nc.tensor.matmul(
                        zp[:], w3b[:, m - 1 : m], h2_prev[:],
                        start=(m == 1), stop=False,
                    )
                h2_prev = h2
            nc.tensor.matmul(
                zp[:], w3b[:, KH - 1 : KH], h2_prev[:],
                start=False, stop=True,
            )

            mb = sm_pool.tile([1, NBLK], F32, name=f"mb_{b}", tag="mb", bufs=1)
            nc.sync.dma_start(mb[:], mb_ext[b : b + 1, :])
            zr = sm_pool.tile([1, NBLK], F32, name=f"zr_{b}", tag="zr", bufs=1)
            nc.scalar.activation(zr[:], zp[:], AF.Identity, bias=b3t[:])
            nc.vector.tensor_add(zr[:], zr[:], mb[:])
            nc.sync.dma_start(z_ext[b : b + 1, :], zr[:])

            vf_l2_chunk(2 * b)
            vf_l2_chunk(2 * b + 1)

        # ---- vf tail: bias+relu rows, L3 dot on DVE --------------------
        # row constants borrow wstage slots (free once the vw2 stream ends)
        vb2row = wstage.tile([B_LOC, H], F32, name="vb2row", tag="wstage")
        vw3row = wstage.tile([B_LOC, H], F32, name="vw3row", tag="wstage")
        for g in range(B_LOC):
            nc.sync.dma_start(vb2row[g : g + 1, :], vb2_ext[None, :])
            nc.sync.dma_start(vw3row[g : g + 1, :], vw3_ext.rearrange("h one -> one h"))
        nc.vector.tensor_add(vacc2[:], vacc2[:], vb2row[:])
        nc.scalar.activation(vacc2[:], vacc2[:], AF.Relu)
        nc.vector.tensor_mul(vacc2[:], vacc2[:], vw3row[:])
        v0 = sm_pool.tile([B_LOC, 1], F32, name="v0", tag="v_sb", bufs=1)
        nc.vector.reduce_sum(v0[:], vacc2[:], axis=mybir.AxisListType.X)
        v_sb = sm_pool.tile([B_LOC, 1], F32, name="v_sb", tag="v_sb2", bufs=1)
        nc.scalar.activation(v_sb[:], v0[:], AF.Identity, bias=vb3col[:])
        nc.sync.dma_start(v_ext[:], v_sb[:])

    nc.compile()
    return nc


_CACHE = threading.Lock()
_NC = None


def _get_nc():
    global _NC
    with _CACHE:
        if _NC is None:
            _NC = _build()
    return _NC


def kernel(**inputs):
    x = np.ascontiguousarray(np.asarray(inputs["x"], dtype=np.float32))
    mask = np.asarray(inputs["action_mask"])
    mb = np.where(mask, np.float32(0.0), np.float32(-np.inf)).astype(np.float32)
    mb = np.ascontiguousarray(mb.reshape(N_CORES * B_LOC, NBLK))

    def f32(name):
        return np.ascontiguousarray(np.asarray(inputs[name], dtype=np.float32))

    shared = {
        "w1": f32("mlp_W1"), "b1": f32("mlp_b1"),
        "w2": f32("mlp_W2"), "b2": f32("mlp_b2"),
        "w3": f32("mlp_W3"), "b3": f32("mlp_b3"),
        "vw1": f32("vf_W1"), "vb1": f32("vf_b1"),
        "vw2": f32("vf_W2"), "vb2": f32("vf_b2"),
        "vw3": f32("vf_W3"), "vb3": f32("vf_b3"),
    }
    in_maps = []
    for c in range(N_CORES):
        m = dict(shared)
        m["x"] = x[c * NODES : (c + 1) * NODES]
        m["mb"] = mb[c * B_LOC : (c + 1) * B_LOC]
        in_maps.append(m)

    nc = _get_nc()
    res = run_bass_kernel_spmd(nc, in_maps, core_ids=list(range(N_CORES)))
    z = np.concatenate([res.results[c]["z"] for c in range(N_CORES)], axis=0)
    v = np.concatenate([res.results[c]["v"] for c in range(N_CORES)], axis=0)
    return z, v


# revision 42
# speedup vs baseline: 1.1891x; 1.0174x over previous
"""Trainium2 Bass kernel for nn_ActorCriticReadOut.

Problem (hardcoded shapes): B=64 graphs x 512 nodes, D=512, H=2048.
  z[g, n]  = where(mask, MLP_3(x)[g*512+n], -inf)        -> [64, 512]
  v[g]     = MLP_vf(mean_n x[g*512+n])                   -> [64, 1]

Sharding: data-parallel over graphs. 8 cores x 8 graphs (4096 nodes) each;
MLP weights replicated. Everything is local per core; host concatenates.

Per-core device algorithm (transposed-activation layout, bf16 compute
with f32 PSUM accumulation):
  phase A (traced first): stream x once (8MB), PE-transpose every
      128x128 tile, DVE-copy psum -> resident xT (bf16, 32KB/part for
      all 8 graphs), and DVE-reduce each xT chunk over its free (node)
      axis to build the segment means directly in transposed layout.
      This unblocks the value MLP ~35us in, so its 20MB of weights
      stream overlapped with the main phase.
  value MLP: all weights streamed (f32 stage -> bf16 cast), no
      residency. L1: k-outer, one-shot matmuls into a packed psum bank
      + DVE accumulate (matmul start=True clears has_written for the
      WHOLE bank, so interleaved accumulation regions are illegal).
      L2: row-form (stationary = h1v column chunk [128, 8], moving =
      weight rows, N=512) -> h2 rows [8, 2048] accumulated in SBUF.
      L3 on DVE: in-place row * w3-row multiply, reduce over free, +vb3.
  main MLP (per 512-node block == one graph, no DMA in the loop):
      L1/L2 bf16 (resident bf16 W1+W2), relu+bias fused into the
      PSUM->SBUF copy on ScalarE; L3 (W3 column stationary, M=1)
      interleaved into L2's m-loop; epilogue adds b3 + additive mask
      row (0/-inf) and DMAs the z row out.
"""

import threading
from contextlib import ExitStack

import numpy as np

import concourse.tile as tile
from concourse import bacc, mybir
from concourse.bass_utils import run_bass_kernel_spmd
from concourse.masks import make_identity

F32 = mybir.dt.float32
BF16 = mybir.dt.bfloat16
AF = mybir.ActivationFunctionType

N_CORES = 8
B_LOC = 8            # graphs per core
NBLK = 512           # nodes per graph (= node block)
D = 512
H = 2048
NODES = B_LOC * NBLK  # 4096
KD = D // 128         # 4 contraction chunks for D
KH = H // 128         # 16 contraction chunks for H
NSUB = NBLK // 128    # 4 node sub-chunks per block
NH = H // 512         # 4 n-slices of H for row-form matmuls


def _build():
    nc = bacc.Bacc(name="actor_critic_readout")

    x_ext = nc.declare_dram_parameter("x", [NODES, D], F32, isOutput=False)
    mb_ext = nc.declare_dram_parameter("mb", [B_LOC, NBLK], F32, isOutput=False)
    w1_ext = nc.declare_dram_parameter("w1", [D, H], F32, isOutput=False)
    b1_ext = nc.declare_dram_parameter("b1", [H], F32, isOutput=False)
    w2_ext = nc.declare_dram_parameter("w2", [H, H], F32, isOutput=False)
    b2_ext = nc.declare_dram_parameter("b2", [H], F32, isOutput=False)
    w3_ext = nc.declare_dram_parameter("w3", [H, 1], F32, isOutput=False)
    b3_ext = nc.declare_dram_parameter("b3", [1], F32, isOutput=False)
    vw1_ext = nc.declare_dram_parameter("vw1", [D, H], F32, isOutput=False)
    vb1_ext = nc.declare_dram_parameter("vb1", [H], F32, isOutput=False)
    vw2_ext = nc.declare_dram_parameter("vw2", [H, H], F32, isOutput=False)
    vb2_ext = nc.declare_dram_parameter("vb2", [H], F32, isOutput=False)
    vw3_ext = nc.declare_dram_parameter("vw3", [H, 1], F32, isOutput=False)
    vb3_ext = nc.declare_dram_parameter("vb3", [1], F32, isOutput=False)
    z_ext = nc.declare_dram_parameter("z", [B_LOC, NBLK], F32, isOutput=True)
    v_ext = nc.declare_dram_parameter("v", [B_LOC, 1], F32, isOutput=True)

    with ExitStack() as ctx:
        tc = ctx.enter_context(tile.TileContext(nc))
        const = ctx.enter_context(tc.tile_pool(name="const", bufs=1))
        wres = ctx.enter_context(tc.tile_pool(name="wres", bufs=1))
        wstage = ctx.enter_context(tc.tile_pool(name="wstage", bufs=3))
        xs_pool = ctx.enter_context(tc.tile_pool(name="xs", bufs=6))
        xta_pool = ctx.enter_context(tc.tile_pool(name="xta", bufs=1))
        h1_pool = ctx.enter_context(tc.tile_pool(name="h1", bufs=17))
        h2_pool = ctx.enter_context(tc.tile_pool(name="h2", bufs=2))
        sm_pool = ctx.enter_context(tc.tile_pool(name="sm", bufs=2))
        vf_pool = ctx.enter_context(tc.tile_pool(name="vf", bufs=1))
        p_l1 = ctx.enter_context(tc.tile_pool(name="p_l1", bufs=3, space="PSUM"))
        # "aux" (1 bank): per-block z rows. "pv" (1 bank): one-shot vf banks.
        p_aux = ctx.enter_context(tc.tile_pool(name="p_aux", bufs=1, space="PSUM"))
        p_vf = ctx.enter_context(tc.tile_pool(name="p_vf", bufs=1, space="PSUM"))

        ident = const.tile([128, 128], BF16, name="ident")
        make_identity(nc, ident)

        # ---- phase A: x -> xT (bf16 resident) + segment sums -----------
        hmsum = [
            const.tile([128, B_LOC], F32, name=f"hmsum_{k}", tag=f"hmsum_{k}")
            for k in range(KD)
        ]
        xta = [[None] * KD for _ in range(B_LOC)]
        with tc.tile_pool(name="p_tp", bufs=2, space="PSUM") as p_tp:
            for b in range(B_LOC):
                xs_tiles = []
                for j in range(NSUB):
                    xs = xs_pool.tile([128, D], F32, name=f"x_{b}_{j}", tag="xs")
                    row = b * NBLK + j * 128
                    nc.sync.dma_start(xs[:], x_ext[row : row + 128, :])
                    xb = xs_pool.tile([128, D], BF16, name=f"xb_{b}_{j}", tag="xb")
                    nc.scalar.copy(xb[:], xs[:])
                    xs_tiles.append(xb)
                for k in range(KD):
                    tp = p_tp.tile([128, NBLK], BF16, name=f"tp_{b}_{k}", tag="tp")
                    for j in range(NSUB):
                        nc.tensor.transpose(
                            tp[:, j * 128 : (j + 1) * 128],
                            xs_tiles[j][:, k * 128 : (k + 1) * 128],
                            ident[:],
                        )
                    t = xta_pool.tile(
                        [128, NBLK], BF16, name=f"xt_{b}_{k}", tag=f"xt_{b}_{k}"
                    )
                    nc.vector.tensor_copy(t[:], tp[:])
                    xta[b][k] = t
                    nc.vector.reduce_sum(
                        hmsum[k][:, b : b + 1], t[:], axis=mybir.AxisListType.X
                    )
        p_l2 = ctx.enter_context(tc.tile_pool(name="p_l2", bufs=3, space="PSUM"))
        # h_meanT chunks, bf16, scale 1/NBLK
        hmb = []
        for k in range(KD):
            t = const.tile([128, B_LOC], BF16, name=f"hmb_{k}", tag=f"hmb_{k}")
            nc.scalar.activation(t[:], hmsum[k][:], AF.Copy, scale=1.0 / NBLK)
            hmb.append(t)

        # ---- small constants -------------------------------------------
        b1t = const.tile([128, KH], F32, name="b1t")
        nc.sync.dma_start(b1t[:], b1_ext.rearrange("(m p) -> p m", p=128))
        b2t = const.tile([128, KH], F32, name="b2t")
        nc.sync.dma_start(b2t[:], b2_ext.rearrange("(m p) -> p m", p=128))
        vb1t = const.tile([128, KH], F32, name="vb1t")
        nc.sync.dma_start(vb1t[:], vb1_ext.rearrange("(m p) -> p m", p=128))
        b3t = const.tile([1, 1], F32, name="b3t")
        nc.sync.dma_start(b3t[:], b3_ext[None, :])
        vb3col = const.tile([B_LOC, 1], F32, name="vb3col")
        for g in range(B_LOC):
            nc.sync.dma_start(vb3col[g : g + 1, :], vb3_ext[None, :])
        w3f = const.tile([128, KH], F32, name="w3f")
        nc.sync.dma_start(w3f[:], w3_ext.rearrange("(k p) one -> p (k one)", p=128))
        w3b = const.tile([128, KH], BF16, name="w3b")
        nc.vector.tensor_copy(w3b[:], w3f[:])

        # ---- resident weights (casts on ScalarE to keep DVE clear) -----
        w1 = []
        for k in range(KD):
            stage = wstage.tile([128, H], F32, name=f"w1s_{k}", tag="wstage")
            nc.sync.dma_start(stage[:], w1_ext[k * 128 : (k + 1) * 128, :])
            t = wres.tile([128, H], BF16, name=f"w1_{k}", tag=f"w1_{k}")
            nc.scalar.copy(t[:], stage[:])
            w1.append(t)
        w2 = []
        for k in range(KH):
            stage = wstage.tile([128, H], F32, name=f"w2s_{k}", tag="wstage")
            nc.sync.dma_start(stage[:], w2_ext[k * 128 : (k + 1) * 128, :])
            t = wres.tile([128, H], BF16, name=f"w2_{k}", tag=f"w2_{k}")
            nc.scalar.copy(t[:], stage[:])
            w2.append(t)

        # ---- vf L1: streamed vw1, k-outer, SBUF accumulate -------------
        h1acc = const.tile([128, KH, B_LOC], F32, name="h1acc")
        for k in range(KD):
            stage = wstage.tile([128, H], F32, name=f"vw1s_{k}", tag="wstage")
            nc.sync.dma_start(stage[:], vw1_ext[k * 128 : (k + 1) * 128, :])
            sbf = wstage.tile([128, H], BF16, name=f"vw1b_{k}", tag="wstageb")
            nc.vector.tensor_copy(sbf[:], stage[:])
            pk = p_vf.tile([128, KH, B_LOC], F32, name=f"pv1_{k}", tag="pv")
            for m in range(KH):
                nc.tensor.matmul(
                    pk[:, m, :],
                    sbf[:, m * 128 : (m + 1) * 128],
                    hmb[k][:],
                    start=True,
                    stop=True,
                )
            if k == 0:
                nc.vector.tensor_copy(h1acc[:], pk[:])
            else:
                nc.vector.tensor_add(h1acc[:], h1acc[:], pk[:])
        h1v = []
        for m in range(KH):
            t = vf_pool.tile([128, B_LOC], BF16, name=f"h1v_{m}", tag=f"h1v_{m}")
            nc.scalar.activation(t[:], h1acc[:, m, :], AF.Relu, bias=vb1t[:, m : m + 1])
            h1v.append(t)

        # ---- vf L2: row-form, streamed vw2, SBUF accumulate ------------
        vacc2 = const.tile([B_LOC, H], F32, name="vacc2")

        def vf_l2_chunk(k):
            stage = wstage.tile([128, H], F32, name=f"vw2s_{k}", tag="wstage")
            nc.sync.dma_start(stage[:], vw2_ext[k * 128 : (k + 1) * 128, :])
            sbf = wstage.tile([128, H], BF16, name=f"vw2b_{k}", tag="wstageb")
            nc.vector.tensor_copy(sbf[:], stage[:])
            for n in range(NH):
                pb = p_vf.tile([B_LOC, 512], F32, name=f"pv2_{k}_{n}", tag="pv")
                nc.tensor.matmul(
                    pb[:],
                    h1v[k][:],
                    sbf[:, n * 512 : (n + 1) * 512],
                    start=True,
                    stop=True,
                )
                sl = vacc2[:, n * 512 : (n + 1) * 512]
                if k == 0:
                    nc.vector.tensor_copy(sl, pb[:])
                else:
                    nc.vector.tensor_add(sl, sl, pb[:])

        # ---- main MLP: one graph (=512-node block) at a time -----------
        for b in range(B_LOC):
            h1 = []
            for m in range(KH):
                ps = p_l1.tile([128, NBLK], F32, name=f"pl1_{b}_{m}", tag="l1")
                for k in range(KD):
                    nc.tensor.matmul(
                        ps[:],
                        w1[k][:, m * 128 : (m + 1) * 128],
                        xta[b][k][:],
                        start=(k == 0),
                        stop=(k == KD - 1),
                    )
                t = h1_pool.tile([128, NBLK], BF16, name=f"h1_{b}_{m}", tag="h1")
                nc.scalar.activation(t[:], ps[:], AF.Relu, bias=b1t[:, m : m + 1])
                h1.append(t)

            zp = p_aux.tile([1, NBLK], F32, name=f"zp_{b}", tag="aux")
            h2_prev = None
            for m in range(KH):
                ps = p_l2.tile([128, NBLK], F32, name=f"pl2_{b}_{m}", tag="l2")
                for k in range(KH):
                    nc.tensor.matmul(
                        ps[:],
                        w2[k][:, m * 128 : (m + 1) * 128],
                        h1[k][:],
                        start=(k == 0),
                        stop=(k == KH - 1),
                    )
                h2 = h2_pool.tile([128, NBLK], BF16, name=f"h2_{b}_{m}", tag="h2")
                nc.scalar.activation(h2[:], ps[:], AF.Relu, bias=b2t[:, m : m + 1])
                # z matmul for the PREVIOUS chunk: its h2 is long since
                # written, so the PE never waits on the ScalarE relu here.
                if h2_prev is not None:
                    nc.tensor.matmul(
                        zp[:], w3b[:, m - 1 : m], h2_prev[:],
                        start=(m == 1), stop=False,
                    )
                h2_prev = h2
            nc.tensor.matmul(
                zp[:], w3b[:, KH - 1 : KH], h2_prev[:],
                start=False, stop=True,
            )

            mb = sm_pool.tile([1, NBLK], F32, name=f"mb_{b}", tag="mb", bufs=1)
            nc.sync.dma_start(mb[:], mb_ext[b : b + 1, :])
            zr = sm_pool.tile([1, NBLK], F32, name=f"zr_{b}", tag="zr", bufs=1)
            nc.scalar.activation(zr[:], zp[:], AF.Identity, bias=b3t[:])
            nc.vector.tensor_add(zr[:], zr[:], mb[:])
            nc.sync.dma_start(z_ext[b : b + 1, :], zr[:])

            vf_l2_chunk(2 * b)
            vf_l2_chunk(2 * b + 1)

        # ---- vf tail: bias+relu rows, L3 dot on DVE --------------------
        # row constants borrow wstage slots (free once the vw2 stream ends)
        vb2row = wstage.tile([B_LOC, H], F32, name="vb2row", tag="wstage")
        vw3row = wstage.tile([B_LOC, H], F32, name="vw3row", tag="wstage")
        for g in range(B_LOC):
            nc.sync.dma_start(vb2row[g : g + 1, :], vb2_ext[None, :])
            nc.sync.dma_start(vw3row[g : g + 1, :], vw3_ext.rearrange("h one -> one h"))
        nc.vector.tensor_add(vacc2[:], vacc2[:], vb2row[:])
        nc.scalar.activation(vacc2[:], vacc2[:], AF.Relu)
        nc.vector.tensor_mul(vacc2[:], vacc2[:], vw3row[:])
        v0 = sm_pool.tile([B_LOC, 1], F32, name="v0", tag="v_sb", bufs=1)
        nc.vector.reduce_sum(v0[:], vacc2[:], axis=mybir.AxisListType.X)
        v_sb = sm_pool.tile([B_LOC, 1], F32, name="v_sb", tag="v_sb2", bufs=1)
        nc.scalar.activation(v_sb[:], v0[:], AF.Identity, bias=vb3col[:])
        nc.sync.dma_start(v_ext[:], v_sb[:])

    nc.compile()
    return nc


_CACHE = threading.Lock()
_NC = None


def _get_nc():
    global _NC
    with _CACHE:
        if _NC is None:
            _NC = _build()
    return _NC


def kernel(**inputs):
    x = np.ascontiguousarray(np.asarray(inputs["x"], dtype=np.float32))
    mask = np.asarray(inputs["action_mask"])
    mb = np.where(mask, np.float32(0.0), np.float32(-np.inf)).astype(np.float32)
    mb = np.ascontiguousarray(mb.reshape(N_CORES * B_LOC, NBLK))

    def f32(name):
        return np.ascontiguousarray(np.asarray(inputs[name], dtype=np.float32))

    shared = {
        "w1": f32("mlp_W1"), "b1": f32("mlp_b1"),
        "w2": f32("mlp_W2"), "b2": f32("mlp_b2"),
        "w3": f32("mlp_W3"), "b3": f32("mlp_b3"),
        "vw1": f32("vf_W1"), "vb1": f32("vf_b1"),
        "vw2": f32("vf_W2"), "vb2": f32("vf_b2"),
        "vw3": f32("vf_W3"), "vb3": f32("vf_b3"),
    }
    in_maps = []
    for c in range(N_CORES):
        m = dict(shared)
        m["x"] = x[c * NODES : (c + 1) * NODES]
        m["mb"] = mb[c * B_LOC : (c + 1) * B_LOC]
        in_maps.append(m)

    nc = _get_nc()
    res = run_bass_kernel_spmd(nc, in_maps, core_ids=list(range(N_CORES)))
    z = np.concatenate([res.results[c]["z"] for c in range(N_CORES)], axis=0)
    v = np.concatenate([res.results[c]["v"] for c in range(N_CORES)], axis=0)
    return z, v
